# revision 11
# baseline (speedup 1.0000x reference)
"""nn_CGBlock Trainium2 kernel v2: f16 I/O, dual-layout loads, DVE-lean.

Data-parallel over batch: 8 NeuronCores x 2 batches each.

Host packs per h-block (4 rows x 128 w = 512 px) one [128, 2048] f16 tile:
  cols    0: 512  x0  ch-major (partition=c 0..127,  free=(t,w))
  cols  512:1024  x1  ch-major (partition=c-128,     free=(t,w))
  cols 1024:2048  xpx px-major (partition=w,         free=(t,c))
Output per block: [128, 1024] f16 (o0 | o1 ch-major), host un-permutes.

Per-block engine split (steady state ~3.2us):
  SP  : 1 HWDGE load (4KB/part), 1 HWDGE store (2KB/part)
  ACT : exp0, exp1 (f16 2x); out0/out1 PSUM->f16 copies; num copy; zT copy
  DVE : recip(prev); 32x max8 (top-8 per (tile,group) window)  <- bottleneck
  Pool: xe0, xe1 = x*e; ymul y = num * (1/s) into z4 y-slots
  PE  : x-accum (ident matmul, start) + delta matmul (stop) -> d = x + delta;
        16 tiny sn matmuls (s, num per (px,group)); 4 z4->zT transposes
"""

from contextlib import ExitStack

import numpy as np

import concourse.bass as bass
import concourse.mybir as mybir
from concourse.bass_utils import run_bass_kernel_spmd

F32 = mybir.dt.float32
F16 = mybir.dt.float16
F8 = mybir.dt.float8e4
NPF16 = np.float16


G = 8
K = 4
ZDIM = 72  # 8 y + 8 groups * 8 max-slots

NCORES = 8
B, C, H, W = 16, 256, 128, 128
NB = B // NCORES
HBLK = 4
NBLK0 = NB * (H // HBLK)  # 64 blocks per core
P = HBLK * W              # 512 px per block

XD = 6   # xall ring
OD = 3   # oall ring


def _build_consts(soft_w1, soft_w2, top_w1, top_w2, r):
    soft_w1 = np.asarray(soft_w1, np.float32)
    soft_w2 = np.asarray(soft_w2, np.float32)
    top_w1 = np.asarray(top_w1, np.float32)
    top_w2 = np.asarray(top_w2, np.float32)
    r = np.asarray(r, np.float32)

    w = np.exp(r - r.max())
    w = w / w.sum()
    rt, rs = np.float32(w[0]), np.float32(w[1])

    w2eff = np.zeros((2, ZDIM, C // 2), np.float32)
    for g in range(G):
        for hf in range(2):
            cols = slice(hf * (C // 2), (hf + 1) * (C // 2))
            w2eff[hf, g, :] = rs * soft_w2[cols, g]
            for k in range(K):
                w2eff[hf, 8 + 8 * g + k, :] = rt * top_w2[cols, g] * top_w1[g, k]
    w2eff = np.ascontiguousarray(w2eff.astype(NPF16))

    masks = np.zeros((2, 128, 8), np.float32)
    for hf in range(2):
        for j in range(4):
            rows = slice(j * 32, (j + 1) * 32)
            masks[hf, rows, j] = 1.0
            masks[hf, rows, 4 + j] = soft_w1[hf * 4 + j, :]
    masks = np.ascontiguousarray(masks.astype(NPF16))

    ident = np.eye(128, dtype=NPF16)
    return {"w2eff": w2eff, "masks": masks, "ident": ident}


def _prep_x(x_core):
    """[NB, C, H, W] f32 -> [NBLK0, 128, 2048] f16 (x0 | x1 | xpx)."""
    nb = x_core.shape[0]
    nblk = nb * (x_core.shape[2] // HBLK)
    xh = np.asarray(x_core, NPF16)
    A = xh.reshape(nb, C, -1, HBLK, W)            # b, c, hb, t, w
    X = np.empty((nb, A.shape[2], 128, 2048), NPF16)
    x01 = A.transpose(0, 2, 1, 3, 4).reshape(nb, A.shape[2], C, HBLK * W)
    X[..., 0:512] = x01[:, :, 0:128, :]
    X[..., 512:1024] = x01[:, :, 128:256, :]
    X[..., 1024:2048] = A.transpose(0, 2, 4, 3, 1).reshape(
        nb, A.shape[2], W, HBLK * C)
    return np.ascontiguousarray(X.reshape(nblk, 128, 2048))


def _unprep_out(o_all, nb=NB, nh=H):
    """[NBLK0, 128, 1024] f16 -> [nb, C, nh, W] f32."""
    nhb = nh // HBLK
    O = o_all.reshape(nb, nhb, 128, 2, HBLK, W)    # b, hb, c, half, t, w
    O = O.transpose(0, 3, 2, 1, 4, 5)              # b, half, c, hb, t, w
    return np.ascontiguousarray(O.astype(np.float32).reshape(nb, C, nh, W))


def _build_kernel(NBLKC=NBLK0, loops=1):
    nc = bass.Bass("TRN2", target_bir_lowering=False, debug=False)

    x_d = nc.dram_tensor("xin", [NBLKC, 128, 2048], F16,
                         kind="ExternalInput").ap()
    w2eff_d = nc.dram_tensor("w2eff", [2, ZDIM, 128], F16,
                             kind="ExternalInput").ap()
    masks_d = nc.dram_tensor("masks", [2, 128, 8], F16,
                             kind="ExternalInput").ap()
    ident_d = nc.dram_tensor("ident", [128, 128], F16,
                             kind="ExternalInput").ap()
    out_d = nc.dram_tensor("out", [NBLKC, 128, 1024], F16,
                           kind="ExternalOutput").ap()

    NBLK = NBLKC * loops
    Exp = mybir.ActivationFunctionType.Exp

    def blk(i):
        return i % NBLKC

    with ExitStack() as ctx:
        def sb(name, shape, dtype=F32):
            return ctx.enter_context(nc.sbuf_tensor(name, shape, dtype))

        def ps(name, shape, dtype=F32):
            return ctx.enter_context(nc.psum_tensor(name, shape, dtype))

        def sem(name):
            return ctx.enter_context(nc.semaphore(name))

        # constants
        identf = sb("identc", [128, 128], F16)
        mask0 = sb("mask0", [128, 8], F16)
        mask1 = sb("mask1", [128, 8], F16)
        w2e0 = sb("w2e0", [ZDIM, 128], F16)
        w2e1 = sb("w2e1", [ZDIM, 128], F16)

        # rings
        xall = [sb(f"xall{j}", [128, 2048], F16) for j in range(XD)]
        e_all = [sb(f"e_{j}", [128, 2 * P], F16) for j in range(2)]
        xe_all = [sb(f"xe_{j}", [128, 2 * P], F16) for j in range(2)]
        z4 = [sb(f"z4_{j}", [128, HBLK * ZDIM], F16) for j in range(3)]
        sn_sb = [sb(f"sn_{j}", [128, HBLK * 16]) for j in range(2)]
        rcp1 = [sb(f"rcp_{j}", [128, 32]) for j in range(2)]
        zT_sb = [sb(f"zT_{j}", [ZDIM, P], F16) for j in range(2)]
        oall = [sb(f"oall{j}", [128, 1024], F16) for j in range(OD)]

        # psum (8 banks)
        sn_ps = [ps(f"snps{j}", [128, HBLK * 16]) for j in range(2)]
        zT_ps = [ps(f"ztps{j}", [ZDIM, P], F16) for j in range(2)]
        d_ps = [ps(f"dps{j}", [128, 2 * P]) for j in range(2)]

        # semaphores
        s_x = [sem(f"s_x{j}") for j in range(XD)]
        s_st = [sem(f"s_st{j}") for j in range(OD)]
        s_cst = sem("s_cst")
        s_exp = sem("s_exp")   # +1 per exp half       -> 2i+2 after block i
        s_xe = sem("s_xe")     # +1 per xe half        -> 2i+2
        s_sn = sem("s_sn")     # +1 after last sn mm   -> i+1
        s_nc = sem("s_nc")     # +1 after sn copy(i)   -> i+1
        s_rc = sem("s_rc")     # +1 after recip(i)     -> i+1
        s_ym = sem("s_ym")     # +1 after ymul(i)      -> i+1
        s_mx = sem("s_mx")     # +1 after last max8(i) -> i+1
        s_tz = sem("s_tz")     # +1 after last T(z)(i) -> i+1
        s_ztc = sem("s_ztc")   # +1 after zT copy(i)   -> i+1
        s_dl = sem("s_dl")     # +1 after delta1(i)    -> i+1
        s_oc = sem("s_oc")     # +1 after out1 copy(i) -> i+1
        s_xa = sem("s_xa")     # +1 after xacc pair(i) -> i+1

        def snv(i, x):
            return sn_sb[i % 2].ap().rearrange(
                "p (t hf x g) -> p t hf x g", t=HBLK, hf=2, x=2)[:, :, :, x, :]

        def rcv(i):
            return rcp1[i % 2].ap().rearrange(
                "p (t hf g) -> p t hf g", t=HBLK, hf=2)

        def z4y(i):
            return z4[i % 3].ap().rearrange(
                "p (t a hf g) -> p t a hf g", t=HBLK, a=9, hf=2)[:, :, 0, :, :]

        with nc.Block() as block:

            @block.sync
            def _(sync):
                sync.dma_start(identf[:], ident_d[:]).then_inc(s_cst, 16)
                sync.dma_start(mask0[:], masks_d[0]).then_inc(s_cst, 16)
                sync.dma_start(mask1[:], masks_d[1]).then_inc(s_cst, 16)
                sync.dma_start(w2e0[:], w2eff_d[0]).then_inc(s_cst, 16)
                sync.dma_start(w2e1[:], w2eff_d[1]).then_inc(s_cst, 16)
                for i in range(NBLK + 5):
                    p = i - 4
                    if 0 <= p < NBLK:
                        sync.wait_ge(s_oc, p + 1)
                        sync.dma_start(out_d[blk(p)], oall[p % OD][:]) \
                            .then_inc(s_st[p % OD], 16)
                    js = [0, 1, 2] if i == 0 else [i + 2]
                    for j in js:
                        if not (0 <= j < NBLK) or (i > 0 and j < 3):
                            continue
                        if j >= XD:
                            sync.wait_ge(s_xa, j - XD + 1)
                        sync.dma_start(xall[j % XD][:], x_d[blk(j)]) \
                            .then_inc(s_x[j % XD], 16)

            @block.scalar
            def _(scalar):
                for i in range(NBLK + 3):
                    if i < NBLK:
                        scalar.wait_ge(s_x[i % XD], 16 * (i // XD + 1))
                        if i >= 2:
                            scalar.wait_ge(s_sn, i - 1)       # e WAR (PE)
                            scalar.wait_ge(s_xe, i - 1)       # e WAR (Pool)
                        scalar.activation(e_all[i % 2][:],
                                          xall[i % XD][:, 0:1024],
                                          Exp).then_inc(s_exp, 1)
                    p = i - 3
                    if 0 <= p < NBLK:
                        scalar.wait_ge(s_dl, p + 1)
                        if p >= OD:
                            scalar.wait_ge(s_st[p % OD],
                                           16 * ((p - OD) // OD + 1))
                        scalar.copy(oall[p % OD][:], d_ps[p % 2][:]) \
                            .then_inc(s_oc, 1)
                    q = i - 1
                    if 0 <= q < NBLK:
                        scalar.wait_ge(s_sn, q + 1)
                        if q >= 2:
                            scalar.wait_ge(s_ym, q - 1)       # sn_sb WAR
                        scalar.copy(sn_sb[q % 2][:], sn_ps[q % 2][:]) \
                            .then_inc(s_nc, 1)
                    r = i - 2
                    if 0 <= r < NBLK:
                        scalar.wait_ge(s_tz, r + 1)
                        if r >= 2:
                            scalar.wait_ge(s_dl, r - 1)       # zT_sb WAR
                        scalar.copy(zT_sb[r % 2][:], zT_ps[r % 2][:]) \
                            .then_inc(s_ztc, 1)

            @block.vector
            def _(vector):
                for i in range(NBLK + 1):
                    if i < NBLK:
                        vector.wait_ge(s_x[i % XD], 16 * (i // XD + 1))
                        if i >= 3:
                            vector.wait_ge(s_tz, i - 2)       # z4 WAR
                        for t in range(HBLK):
                            for g in range(G):
                                mx = vector.max(
                                    z4[i % 3][:, t * ZDIM + 8 + 8 * g:
                                              t * ZDIM + 16 + 8 * g],
                                    xall[i % XD][:, 1024 + t * 256 + g * 32:
                                                 1024 + t * 256 + g * 32 + 32])
                        mx.then_inc(s_mx, 1)
                    q = i - 1
                    if 0 <= q < NBLK:
                        vector.wait_ge(s_nc, q + 1)
                        if q >= 2:
                            vector.wait_ge(s_ym, q - 1)       # rcp WAR
                        vector.reciprocal(rcv(q), snv(q, 0)).then_inc(s_rc, 1)

            @block.gpsimd
            def _(gpsimd):
                for i in range(NBLK + 1):
                    if i < NBLK:
                        if i >= 2:
                            gpsimd.wait_ge(s_sn, i - 1)       # xe WAR (PE)
                        gpsimd.wait_ge(s_exp, i + 1)
                        gpsimd.tensor_mul(xe_all[i % 2][:],
                                          xall[i % XD][:, 0:1024],
                                          e_all[i % 2][:]).then_inc(s_xe, 1)
                    q = i - 1
                    if 0 <= q < NBLK:
                        gpsimd.wait_ge(s_rc, q + 1)
                        if q >= 3:
                            gpsimd.wait_ge(s_tz, q - 2)       # z4 WAR
                        gpsimd.tensor_mul(z4y(q), snv(q, 1), rcv(q)) \
                            .then_inc(s_ym, 1)

            @block.tensor
            def _(tensor):
                tensor.wait_ge(s_cst, 80)
                for i in range(NBLK + 2):
                    p = i - 2
                    if 0 <= p < NBLK:
                        if p >= 2:
                            tensor.wait_ge(s_oc, p - 1)       # d_ps WAR
                        tensor.matmul(d_ps[p % 2][:, 0:512], identf[:],
                                      xall[p % XD][:, 0:512],
                                      start=True, stop=False,
                                      skip_group_check=True)
                        tensor.matmul(d_ps[p % 2][:, 512:1024], identf[:],
                                      xall[p % XD][:, 512:1024],
                                      start=True, stop=False,
                                      skip_group_check=True).then_inc(s_xa, 1)
                    if i < NBLK:
                        if i >= 2:
                            tensor.wait_ge(s_nc, i - 1)       # sn_ps WAR
                        tensor.wait_ge(s_exp, i + 1)
                        for t in range(HBLK):
                            tensor.matmul(
                                sn_ps[i % 2][:, 16 * t:16 * t + 4],
                                e_all[i % 2][:, 128 * t:128 * t + 128],
                                mask0[:, 0:4], start=True, stop=True)
                            tensor.matmul(
                                sn_ps[i % 2][:, 16 * t + 8:16 * t + 12],
                                e_all[i % 2][:, 512 + 128 * t:512 + 128 * t
                                              + 128],
                                mask1[:, 0:4], start=True, stop=True)
                        tensor.wait_ge(s_xe, i + 1)
                        for t in range(HBLK):
                            tensor.matmul(
                                sn_ps[i % 2][:, 16 * t + 4:16 * t + 8],
                                xe_all[i % 2][:, 128 * t:128 * t + 128],
                                mask0[:, 4:8], start=True, stop=True)
                            mm = tensor.matmul(
                                sn_ps[i % 2][:, 16 * t + 12:16 * t + 16],
                                xe_all[i % 2][:, 512 + 128 * t:512 + 128 * t
                                               + 128],
                                mask1[:, 4:8], start=True, stop=True)
                        mm.then_inc(s_sn, 1)
                    if 0 <= p < NBLK:
                        tensor.wait_ge(s_ztc, p + 1)
                        tensor.matmul(d_ps[p % 2][:, 0:512], w2e0[:],
                                      zT_sb[p % 2][:], start=False, stop=True,
                                      skip_group_check=True)
                        tensor.matmul(d_ps[p % 2][:, 512:1024], w2e1[:],
                                      zT_sb[p % 2][:], start=False, stop=True,
                                      skip_group_check=True).then_inc(s_dl, 1)
                    r = i - 1
                    if 0 <= r < NBLK:
                        if r >= 2:
                            tensor.wait_ge(s_ztc, r - 1)      # zT_ps WAR
                        tensor.wait_ge(s_ym, r + 1)
                        tensor.wait_ge(s_mx, r + 1)
                        for t in range(HBLK):
                            mm = tensor.transpose(
                                zT_ps[r % 2][:, 128 * t:128 * t + 128],
                                z4[r % 3][:, ZDIM * t:ZDIM * t + ZDIM],
                                identf[:])
                        mm.then_inc(s_tz, 1)

    return nc


_NC_CACHE = {}


def _get_nc(loops=1):
    if loops not in _NC_CACHE:
        _NC_CACHE[loops] = _build_kernel(loops=loops)
    return _NC_CACHE[loops]


def _make_in_maps(x, soft_w1, soft_w2, top_w1, top_w2, r):
    x = np.asarray(x, np.float32)
    consts = _build_consts(soft_w1, soft_w2, top_w1, top_w2, r)
    in_maps = []
    for i in range(NCORES):
        in_maps.append({
            "xin": _prep_x(x[i * NB:(i + 1) * NB]),
            "w2eff": consts["w2eff"],
            "masks": consts["masks"],
            "ident": consts["ident"],
        })
    return in_maps


def kernel(x, soft_w1, soft_w2, top_w1, top_w2, r, _trace=False, _tmpdir=None,
           _loops=1):
    assert np.asarray(x).shape == (B, C, H, W)
    in_maps = _make_in_maps(x, soft_w1, soft_w2, top_w1, top_w2, r)
    nc = _get_nc(_loops)
    res = run_bass_kernel_spmd(nc, in_maps, core_ids=list(range(NCORES)),
                               trace=_trace, tmpdir=_tmpdir)
    out = np.concatenate(
        [_unprep_out(np.asarray(res.results[i]["out"]).reshape(NBLK0, 128,
                                                               1024))
         for i in range(NCORES)], axis=0)
    if _trace:
        return out, res
    return out
